# revision 51
# baseline (speedup 1.0000x reference)
"""Causal self-attention (B=2, T=2048, C=1024, H=16) on 8 TRN2 NeuronCores.

Sharding: core = b*4 + hg  (data parallel over batch, tensor parallel over
4 head-groups of 4 heads). Each core computes its head-group's attention and
a partial output projection; the host sums the 4 partials per batch and adds
b_proj.

Per-core device program (v3 - phase-fused):
  - x/Wqk/Wv/q/k/v/p in fp16 (walrus rejects mixed 16/32-bit matmuls, so the
    whole attention path is 16-bit); y is float32r; PSUM accumulation fp32;
    partial outputs returned in fp16 (halves the 8MB output DMA).
  - xT streams in 512-column chunks so the fused ch-major qkv loop starts as
    soon as the first block lands; x@Wv touches only the 3 useful column
    ranges of the ones-augmented v block (the bias matmul initializes pads).
  - v_aug carries a per-head ones column so the o-matmul accumulates the
    softmax denominator D on PSUM partition {64,0,96,32}[h] while the head's
    v columns land exactly on its yT rows.
  - Attention runs per (head-pair, 512-wide query chunk): both heads share
    one [128,1024] score tile and ONE exp (ACT paces attention, so fewer,
    wider exps matter); o-matmuls trail the scores by one key block so the
    exp latency never stalls the in-order PE. Triangular masks on GPSIMD.
  - Normalization splits: PSUM->SBUF copies + 1/D recips (DVE) run at chunk
    end; the 1/D broadcast matmuls + yT scale-muls are deferred into the
    next chunk's loop. Projection column-tiles are likewise slotted into
    later ACT-paced attention chunks once their query range is normalized,
    leaving only 4 projection tiles after the last attention chunk.
"""

import math

import numpy as np

import concourse.bass as bass
import concourse.bacc as bacc
import concourse.mybir as mybir
from concourse import tile
from concourse.bass_utils import run_bass_kernel_spmd

B, T, C, H = 2, 2048, 1024, 16
HD = C // H   # 64
HPG = 4       # heads per group
NG = 4        # head groups
NCORES = 8

F32 = mybir.dt.float32
F32R = mybir.dt.float32r
BF16 = mybir.dt.bfloat16
F16 = mybir.dt.float16
AF = mybir.ActivationFunctionType
SCALE = 1.0 / math.sqrt(C)  # 1/32

# Per-head layout of the v_aug stationary block: (col offset, width,
# v-column offset within block, ones-column offset within block).
# v columns sit at PSUM rows (h%2)*64..+64; ones column on a 32-aligned row.
V_BLK = [
    (0, 65, 0, 64),      # h0: v@0-63,  D@64
    (65, 128, 64, 0),    # h1: v@64-127, D@0
    (193, 97, 0, 96),    # h2: v@0-63,  D@96
    (290, 128, 64, 32),  # h3: v@64-127, D@32
]
VW = 418  # total v_aug width
DROW = [64, 0, 96, 32]  # PSUM partition of D per head


def build_program(reps=1, qk_bias=False):
    nc = bacc.Bacc()

    xT = nc.dram_tensor("xT", [C, T], F16, kind="ExternalInput")
    wqk = nc.dram_tensor("wqk", [C, 512], F16, kind="ExternalInput")
    bqk = nc.dram_tensor("bqk", [128, 4], F32, kind="ExternalInput")
    wv = nc.dram_tensor("wv", [C, VW], F16, kind="ExternalInput")
    bv = nc.dram_tensor("bv", [1, VW], F16, kind="ExternalInput")
    wp = nc.dram_tensor("wp", [256, 1024], F32R, kind="ExternalInput")
    mask = nc.dram_tensor("mask", [128, 128], F16, kind="ExternalInput")
    ones = nc.dram_tensor("ones", [1, 128], F16, kind="ExternalInput")
    onesf = nc.dram_tensor("onesf", [128, 128], F32R, kind="ExternalInput")
    out = nc.dram_tensor("out", [T, C], F16, kind="ExternalOutput")

    with tile.TileContext(nc) as tc:
        with (
            tc.tile_pool(name="big", bufs=32) as big_pool,
            tc.tile_pool(name="pp", bufs=6) as p_pool,
            tc.tile_pool(name="osb", bufs=3) as o_pool,
            tc.tile_pool(name="wqk", bufs=8) as wqk_pool,
            tc.tile_pool(name="wv", bufs=8) as wv_pool,
            tc.tile_pool(name="qkT", bufs=4) as qkT_pool,
            tc.tile_pool(name="vsb", bufs=16) as v_pool,
            tc.tile_pool(name="yT", bufs=2) as yT_pool,
            tc.tile_pool(name="wp", bufs=2) as wp_pool,
            tc.tile_pool(name="consts", bufs=1) as c_pool,
            tc.tile_pool(name="psA", bufs=2, space="PSUM") as psA,
            tc.tile_pool(name="psB", bufs=2, space="PSUM") as psB,
            tc.tile_pool(name="psC", bufs=2, space="PSUM") as psC,
        ):
          for rep in range(reps):
            # ---- loads. xT comes in 512-col chunks, ch-major, so the qkv
            # loop can start as soon as the first column block lands; wv/wp
            # are deferred behind the ch0 prologue on the gpsimd queue. ----
            d128 = c_pool.tile([128, T], F32, tag="d128")
            r128 = c_pool.tile([128, T], F32R, tag="r128")
            wqk_sb, wv_sb = [], []
            for ct in range(8):
                w_ = wqk_pool.tile([128, 512], F16, tag="wqk", name=f"wqk{ct}")
                weng = nc.scalar if ct % 2 == 0 else nc.sync
                weng.dma_start(w_[:], wqk[ct * 128:(ct + 1) * 128, :])
                wqk_sb.append(w_)
            # gpsimd SWDGE issue costs ~1us per DMA: order by first use.
            bv_sb = c_pool.tile([1, VW], F16, tag="bv")
            nc.gpsimd.dma_start(bv_sb[:], bv[:])
            ones_sb = c_pool.tile([1, 128], F16, tag="ones")
            nc.gpsimd.dma_start(ones_sb[:], ones[:])
            for ct in range(8):
                t_ = wv_pool.tile([128, VW], F16, tag="wv", name=f"wv{ct}")
                nc.gpsimd.dma_start(t_[:], wv[ct * 128:(ct + 1) * 128, :])
                wv_sb.append(t_)
            mask_sb = c_pool.tile([128, 128], F16, tag="mask")
            nc.gpsimd.dma_start(mask_sb[:], mask[:])
            onesf_sb = c_pool.tile([128, 128], F32R, tag="onesf")
            nc.gpsimd.dma_start(onesf_sb[:], onesf[:])
            if qk_bias:
                bqk_sb = c_pool.tile([128, 4], F32, tag="bqk")
                nc.gpsimd.dma_start(bqk_sb[:], bqk[:])
            # xtc[ct][ch] covers xT[ct*128:+128, ch*512:+512]
            xtc = [[None] * 4 for _ in range(8)]
            qdma = [nc.sync, nc.scalar]
            for ch in range(4):
                for ct in range(8):
                    t_ = big_pool.tile([128, 512], F16, tag="big",
                                       name=f"xt{ct}_{ch}")
                    qdma[(ch * 8 + ct) % 2].dma_start(
                        t_[:], xT[ct * 128:(ct + 1) * 128,
                                  ch * 512:(ch + 1) * 512])
                    xtc[ct][ch] = t_
            # wp last: not needed until the first projection tile (~70us)
            wp_sb = []
            for mt in range(2):
                t_ = wp_pool.tile([128, 1024], F32R, tag="wp", name=f"wp{mt}")
                qdma[mt % 2].dma_start(t_[:], wp[mt * 128:(mt + 1) * 128, :])
                wp_sb.append(t_)

            qkT_sb = [
                qkT_pool.tile([128, T], F16, tag="qkT", name=f"qkT{j}")
                for j in range(4)
            ]
            yT_sb = [
                yT_pool.tile([128, T], F32R, tag="yT", name=f"yT{m}")
                for m in range(2)
            ]

            # x@Wv only on columns that hold real v data; the bias matmul
            # (start=True) initializes the full VW span incl. ones/pad cols.
            V_RANGES = [(0, 64), (129, 128), (354, 64)]
            v_sb = [None] * 16

            # ---- emitters ----------------------------------------------
            def emit_qk_jt(ch, jt):
                ps = psB.tile([128, 512], F32, tag="B", name="qk_ps")
                for ct in range(8):
                    nc.tensor.matmul(
                        ps[:, 0:512],
                        wqk_sb[ct][:, jt * 128:(jt + 1) * 128],
                        xtc[ct][ch][:, 0:512],
                        start=(ct == 0),
                        stop=(ct == 7),
                    )
                dst = qkT_sb[jt][:, ch * 512:(ch + 1) * 512]
                if qk_bias:
                    nc.vector.tensor_scalar_add(
                        dst, ps[:, 0:512], bqk_sb[:, jt:jt + 1]
                    )
                elif jt % 2 == 0:
                    nc.vector.tensor_copy(dst, ps[:, 0:512])
                else:
                    nc.scalar.copy(dst, ps[:, 0:512])

            def emit_v_tt(ch, tt):
                ps = psB.tile([128, 512], F32, tag="B", name="v_ps")
                nc.tensor.matmul(
                    ps[:, 0:VW],
                    ones_sb[0:1, 0:128],
                    bv_sb[0:1, 0:VW],
                    start=True,
                    stop=False,
                )
                xblk = (tt % 4) * 128
                for ct in range(8):
                    for ri, (ro, rw) in enumerate(V_RANGES):
                        nc.tensor.matmul(
                            ps[:, ro:ro + rw],
                            xtc[ct][ch][:, xblk:xblk + 128],
                            wv_sb[ct][:, ro:ro + rw],
                            start=False,
                            stop=(ct == 7 and ri == len(V_RANGES) - 1),
                            skip_group_check=True,
                        )
                t_ = v_pool.tile([128, VW], F16, tag="v", name=f"v{tt}")
                eng = nc.scalar.copy if ch == 0 else (
                    nc.vector.tensor_copy if tt % 2 == 0 else nc.scalar.copy)
                eng(t_[:], ps[:, 0:VW])
                v_sb[tt] = t_

            def emit_proj_tt(tt):
                o_sb = o_pool.tile([128, 1024], F16, tag="o", name=f"o_sb{tt}")
                for nch in range(2):
                    ps = psC.tile([128, 512], F32, tag="C", name="pj_ps")
                    for mt in range(2):
                        nc.tensor.matmul(
                            ps[:, 0:512],
                            yT_sb[mt][:, tt * 128:(tt + 1) * 128],
                            wp_sb[mt][:, nch * 512:(nch + 1) * 512],
                            start=(mt == 0),
                            stop=(mt == 1),
                        )
                    # DVE while attention runs (ACT exp-saturated); the tail
                    # tiles (tt>=12) split DVE/ACT since exps are done by then
                    if tt >= 12 and nch == 1:
                        nc.scalar.copy(
                            o_sb[:, nch * 512:(nch + 1) * 512], ps[:, 0:512]
                        )
                    else:
                        nc.vector.tensor_copy(
                            o_sb[:, nch * 512:(nch + 1) * 512], ps[:, 0:512]
                        )
                qdma[tt % 2].dma_start(out[tt * 128:(tt + 1) * 128, :], o_sb[:])

            def emit_qkv_ch(ch, slots=()):
                """One 512-column block of q/k/v projections; `slots` are
                deferred PE jobs (norm-backs, proj tiles) woven between the
                eight matmul chains to ride out their exp/DVE dependencies."""
                slots = list(slots)
                units = [lambda jt=jt: emit_qk_jt(ch, jt) for jt in (0, 2, 1, 3)]
                units += [lambda tt=tt: emit_v_tt(ch, tt)
                          for tt in range(4 * ch, 4 * ch + 4)]
                for i, u in enumerate(units):
                    u()
                    if slots:
                        slots.pop(0)()
                for s in slots:
                    s()

            # ---- attention chunk: heads of a pair share one [128,1024]
            # score tile and ONE exp; o-matmuls trail by one key block so the
            # exp latency never stalls the in-order PE stream.
            def attn_qc(hp, qc, slots=()):
                slots = list(slots)
                q_tile = qkT_sb[hp]
                k_tile = qkT_sb[2 + hp]
                q0 = qc * 512
                njt = 4 * qc + 4
                o_t = [
                    psB.tile([128, 512], F32, tag="B", name=f"o{hp}_{qc}_{hl}")
                    for hl in range(2)
                ]
                pend = None

                def flush_pend():
                    jp, p_prev = pend
                    Lp = max(0, jp * 128 - q0)
                    for hl in range(2):
                        blk_off, blk_w, _, _ = V_BLK[2 * hp + hl]
                        nc.tensor.matmul(
                            o_t[hl][0:blk_w, Lp:512],
                            v_sb[jp][:, blk_off:blk_off + blk_w],
                            p_prev[:, hl * 512 + Lp:hl * 512 + 512],
                            start=(jp == 0),
                            stop=(jp == njt - 1),
                            skip_group_check=True,
                        )

                for jt in range(njt):
                    L = max(0, jt * 128 - q0)
                    s_ps = psA.tile([128, 1024], F32, tag="A", name="s_ps")
                    for hl in range(2):
                        qrow = hl * 64
                        nc.tensor.matmul(
                            s_ps[:, hl * 512 + L:hl * 512 + 512],
                            k_tile[qrow:qrow + 64, jt * 128:(jt + 1) * 128],
                            q_tile[qrow:qrow + 64, q0 + L:q0 + 512],
                            start=True,
                            stop=True,
                        )
                    p_sb = p_pool.tile([128, 1024], F16, tag="p", name="p_sb")
                    nc.scalar.activation(
                        p_sb[:, L:1024], s_ps[:, L:1024], AF.Exp, scale=SCALE
                    )
                    if jt >= 4 * qc:  # diagonal block (both heads)
                        nc.gpsimd.tensor_mul(
                            p_sb[:, L:L + 128], p_sb[:, L:L + 128], mask_sb[:]
                        )
                        nc.gpsimd.tensor_mul(
                            p_sb[:, 512 + L:512 + L + 128],
                            p_sb[:, 512 + L:512 + L + 128],
                            mask_sb[:],
                        )
                    if pend is not None:
                        flush_pend()
                    pend = (jt, p_sb)
                    if jt % 2 == 1 and slots:
                        slots.pop(0)()
                flush_pend()
                for s in slots:
                    s()
                # PSUM->SBUF copies + 1/D (DVE); the rb broadcasts + muls are
                # deferred (returned) so this DVE chain never blocks the PE.
                qsl = slice(q0, q0 + 512)
                pa, pb = DROW[2 * hp], DROW[2 * hp + 1]
                last = hp == 1 and qc == 3  # ACT is free after the last exps
                for hl in range(2):
                    h = 2 * hp + hl
                    qrow = hl * 64
                    blk_off, blk_w, v_off, one_off = V_BLK[h]
                    nc.vector.tensor_copy(
                        yT_sb[hp][qrow:qrow + 64, qsl],
                        o_t[hl][v_off:v_off + 64, 0:512],
                    )
                    deng = nc.scalar.copy if last else nc.vector.tensor_copy
                    deng(
                        d128[DROW[h]:DROW[h] + 1, qsl],
                        o_t[hl][one_off:one_off + 1, 0:512],
                    )
                with nc.allow_low_precision(reason="1/D f32r feeds mm"):
                    nc.vector.reciprocal(r128[pa:pa + 1, qsl],
                                         d128[pa:pa + 1, qsl])
                    nc.vector.reciprocal(r128[pb:pb + 1, qsl],
                                         d128[pb:pb + 1, qsl])

                def norm_back():
                    rb = []
                    for pp_ in (pa, pb):
                        t_ = psC.tile([128, 512], F32, tag="C", name="rb")
                        nc.tensor.matmul(
                            t_[:, 0:512],
                            onesf_sb[pp_:pp_ + 1, 0:128],
                            r128[pp_:pp_ + 1, qsl],
                            start=True,
                            stop=True,
                            tile_position=(pp_, 0),
                        )
                        rb.append(t_)
                    nc.vector.tensor_mul(
                        yT_sb[hp][0:64, qsl], yT_sb[hp][0:64, qsl],
                        rb[0][0:64, 0:512],
                    )
                    nc.vector.tensor_mul(
                        yT_sb[hp][64:128, qsl], yT_sb[hp][64:128, qsl],
                        rb[1][64:128, 0:512],
                    )

                return norm_back

            # ---- schedule: attention chunk qc only needs x-columns <= qc,
            # so qkv block ch and attention chunk qc=ch-1 interleave; the
            # PE-heavy qkv chains keep the PE fed while ACT grinds exps.
            def pj(t):
                return lambda: emit_proj_tt(t)

            emit_qkv_ch(0)
            nbA0 = attn_qc(0, 0)
            nbB0 = attn_qc(1, 0, [nbA0])
            emit_qkv_ch(1)
            nbA1 = attn_qc(0, 1, [nbB0])
            nbB1 = attn_qc(1, 1, [nbA1, pj(0)])
            emit_qkv_ch(2)
            nbA2 = attn_qc(0, 2, [nbB1, pj(1), pj(2)])
            nbB2 = attn_qc(1, 2, [nbA2, pj(3), pj(4), pj(5)])
            emit_qkv_ch(3)
            nbA3 = attn_qc(0, 3, [nbB2, pj(6), pj(7), pj(8), pj(9)])
            nbB3 = attn_qc(1, 3, [nbA3, pj(10), pj(11)])
            nbB3()
            for tt in range(12, 16):
                emit_proj_tt(tt)

    if not nc.is_finalized():
        nc.finalize()
    return nc


def host_prep(x, W_attn, b_attn, W_proj):
    bf = np.float16
    x = np.ascontiguousarray(np.asarray(x, np.float32))
    W_attn = np.ascontiguousarray(np.asarray(W_attn, np.float32))
    b_attn = np.ascontiguousarray(np.asarray(b_attn, np.float32))
    W_proj = np.ascontiguousarray(np.asarray(W_proj, np.float32))
    mask = np.triu(np.ones((128, 128), bf))
    ones = np.ones((1, 128), bf)
    onesf = np.ones((128, 128), np.float32)
    per_group = []
    for hg in range(NG):
        heads = [hg * HPG + i for i in range(HPG)]
        wq = np.concatenate([W_attn[:, h * HD:(h + 1) * HD] for h in heads], axis=1)
        wk = np.concatenate(
            [W_attn[:, C + h * HD:C + (h + 1) * HD] for h in heads], axis=1
        )
        wqk_ = np.ascontiguousarray(np.concatenate([wq, wk], axis=1).astype(bf))
        bq = np.concatenate([b_attn[h * HD:(h + 1) * HD] for h in heads])
        bk = np.concatenate([b_attn[C + h * HD:C + (h + 1) * HD] for h in heads])
        bqk_ = np.ascontiguousarray(np.concatenate([bq, bk]).reshape(4, 128).T)
        wv_ = np.zeros((C, VW), np.float32)
        bv_ = np.zeros((1, VW), np.float32)
        for i, h in enumerate(heads):
            blk_off, blk_w, v_off, one_off = V_BLK[i]
            wv_[:, blk_off + v_off:blk_off + v_off + 64] = \
                W_attn[:, 2 * C + h * HD:2 * C + (h + 1) * HD]
            bv_[0, blk_off + v_off:blk_off + v_off + 64] = \
                b_attn[2 * C + h * HD:2 * C + (h + 1) * HD]
            bv_[0, blk_off + one_off] = 1.0
        wp_ = np.ascontiguousarray(
            np.concatenate([W_proj[h * HD:(h + 1) * HD, :] for h in heads], axis=0)
        )
        per_group.append((wqk_, bqk_, wv_.astype(bf), bv_.astype(bf), wp_))
    in_maps = []
    for b in range(B):
        xT_b = np.ascontiguousarray(x[b].T.astype(bf))
        for hg in range(NG):
            wqk_, bqk_, wv_, bv_, wp_ = per_group[hg]
            in_maps.append(
                dict(xT=xT_b, wqk=wqk_, bqk=bqk_, wv=wv_, bv=bv_, wp=wp_,
                     mask=mask, ones=ones, onesf=onesf)
            )
    return in_maps


_prog_cache = {}


def _get_program(qk_bias=False):
    key = ("nc", qk_bias)
    if key not in _prog_cache:
        _prog_cache[key] = build_program(qk_bias=qk_bias)
    return _prog_cache[key]


def run_cores(in_maps, trace=False, qk_bias=False, **kw):
    return run_bass_kernel_spmd(
        _get_program(qk_bias), in_maps, list(range(NCORES)), trace=trace, **kw
    )


def kernel(x, W_attn, b_attn, W_proj, b_proj):
    in_maps = host_prep(x, W_attn, b_attn, W_proj)
    qk_bias = bool(np.any(np.asarray(b_attn, np.float32)[: 2 * C]))
    br = run_cores(in_maps, qk_bias=qk_bias)
    b_proj = np.asarray(b_proj, np.float32)
    y = np.zeros((B, T, C), np.float32)
    for b in range(B):
        acc = np.zeros((T, C), np.float32)
        for hg in range(NG):
            acc += np.asarray(br.results[b * NG + hg]["out"])
        y[b] = acc + b_proj[None, :]
    return y

